# revision 7
# baseline (speedup 1.0000x reference)
"""GAT layer on 8 Trainium2 NeuronCores (Bass/Tile).

Strategy (dst-sharded, no collectives):
- Rows (dst nodes) are partitioned into 8 contiguous ranges of 12500; core k
  owns all edges whose dst row falls in its range, so softmax stats and
  aggregation complete locally and the host just concatenates outputs.
- Phase A (per core): xp = x @ W.T (feature-permuted) and d = xp . dst_attn
  computed on the tensor engine; written to a DRAM table [NPAD, 256] f16 with
  row layout [xp(128) | d(8) | pad]; 512B row stride (dma_gather quantum).
- Phase B: edges sorted by (block of 128 dst rows, src-col bucket); per-edge
  rows gathered from the table with dma_gather (int16 idx => 4 col buckets of
  25600). Logits t = s[row] + d[col]; s[row] arrives as a host-expanded
  sequential stream (s = x @ C_s is 0.1% of FLOPs). w = exp(lrelu(t) - 3)
  (softmax shift-invariance makes the constant bias exact). Aggregation:
  per 128-edge subtile a 0/1 selection matrix Sel[edge, row] is built on the
  vector engine and the tensor engine computes psum[row, :] += Sel.T @
  [w*xp | w], accumulating numerator and denominator in one matmul; the
  flush divides and writes out rows.
"""

import math
import numpy as np

N_NODES = 100000
N_EDGES = 1600000
IN_DIM = 128
H = 8
HD = 16
NEG_SLOPE = 0.2

NCORES = 8
R_CORE = 12500            # dst rows per core
NBLK = 98                 # ceil(12500/128)
RPAD = NBLK * 128         # 12544
TA = 25                   # phase-A nodes per partition per batch
BATCH_NODES = 128 * TA    # 3200
NBATCH = 32
NPAD = NBATCH * BATCH_NODES  # 102400
TROW = 256                # table row stride in f16 elements (512B)
TUSED = 136               # used part of table row: xp(128) + d(8)
BUCKET = 25600
NBUCK = 4
GBLK = 4                  # blocks per gather group
EXP_BIAS = -3.0


def _feature_perm():
    # f' = u*8 + h  <->  f = h*16 + u
    perm = np.empty(IN_DIM, dtype=np.int64)
    for u in range(HD):
        for h in range(H):
            perm[u * H + h] = h * HD + u
    return perm


def host_prep(x, edge_indices, W, src_attn, dst_attn):
    """All host-side preprocessing. Returns (shared inputs, per-core inputs,
    schedule) for the SPMD program."""
    x = np.asarray(x, dtype=np.float32)
    W = np.asarray(W, dtype=np.float32)
    src_attn = np.asarray(src_attn, dtype=np.float32).reshape(H, HD)
    dst_attn = np.asarray(dst_attn, dtype=np.float32).reshape(H, HD)
    ei = np.asarray(edge_indices)
    row = ei[0].astype(np.int32)
    col = ei[1].astype(np.int32)

    perm = _feature_perm()
    W_perm = W[perm]                                  # [128 f', 128 i]
    # WC = [W_perm.T | C_d]: psum cols 0:128 = xp_perm, 128:136 = d
    C_d = np.einsum('hui,hu->ih', W.reshape(H, HD, IN_DIM), dst_attn)  # [i, h]
    WC = np.concatenate([W_perm.T, C_d], axis=1).astype(np.float16)    # [128, 136]
    C_s = np.einsum('hui,hu->ih', W.reshape(H, HD, IN_DIM), src_attn)
    s_all = (x @ C_s).astype(np.float32)              # [N, 8] (0.1% of FLOPs)

    # xT with phase-A batch column permutation:
    # xT_host[:, B*3200 + j*128 + p] = x[B*3200 + p*25 + j, :]
    x_pad = np.zeros((NPAD, IN_DIM), dtype=np.float32)
    x_pad[:N_NODES] = x
    xT = np.ascontiguousarray(
        x_pad.reshape(NBATCH, 128, TA, IN_DIM).transpose(3, 0, 2, 1).reshape(IN_DIM, NPAD)
    ).astype(np.float16)

    iota = np.tile(np.arange(128, dtype=np.float16), (128, 1))

    # --- edge partition & schedule ---
    core = row // R_CORE
    r_loc = row - core * R_CORE
    blk = r_loc >> 7
    buck = col // BUCKET
    # per-core sorted edge lists + counts per (block, bucket)
    cnt = np.zeros((NCORES, NBLK, NBUCK), dtype=np.int64)
    np.add.at(cnt, (core, blk, buck), 1)
    caps = (128 * np.ceil(cnt.max(axis=0) / 128)).astype(np.int64)  # [NBLK, NBUCK]

    # group layout: for each group of GBLK blocks, stream = for B: for b in grp
    groups = []
    off = 0
    for g0 in range(0, NBLK, GBLK):
        bs = list(range(g0, min(g0 + GBLK, NBLK)))
        runs = []   # per bucket: (stream_off, length)
        subtiles = []  # (block, ) per 128-slot subtile in stream order
        seg_off = {}
        for B in range(NBUCK):
            run_off = off
            for b in bs:
                c = int(caps[b, B])
                if c == 0:
                    continue
                seg_off[(b, B)] = off
                subtiles += [b] * (c // 128)
                off += c
            runs.append((run_off, off - run_off, B))
        groups.append(dict(blocks=bs, runs=runs, subtiles=subtiles,
                           seg_off=seg_off, start=runs[0][0], end=off))
    S = off
    assert S % 128 == 0
    S16, S128 = S // 16, S // 128

    # per-core streams
    per_core = []
    order = np.lexsort((buck, blk, core))  # sort edges by (core, blk, buck)
    row_s, col_s = row[order], col[order]
    core_s, blk_s, buck_s = core[order], blk[order], buck[order]
    # start index of each (core, blk, buck) run inside the sorted arrays
    for k in range(NCORES):
        colidx = np.zeros(S, dtype=np.int16)
        rowloc = np.full(S, -1.0, dtype=np.float16)
        sstream = np.zeros((S, 8), dtype=np.float32)
        sel_k = core_s == k
        e_blk = blk_s[sel_k]; e_buck = buck_s[sel_k]
        e_row = row_s[sel_k]; e_col = col_s[sel_k]
        # run boundaries: edges already sorted by (blk, buck)
        key = e_blk.astype(np.int64) * NBUCK + e_buck
        starts = np.searchsorted(key, np.arange(NBLK * NBUCK, dtype=np.int64))
        ends = np.searchsorted(key, np.arange(NBLK * NBUCK, dtype=np.int64), side='right')
        for g in groups:
            for (b, B), o in g["seg_off"].items():
                a, e = starts[b * NBUCK + B], ends[b * NBUCK + B]
                n = e - a
                if n == 0:
                    continue
                colidx[o:o + n] = (e_col[a:e] - B * BUCKET).astype(np.int16)
                rowloc[o:o + n] = (e_row[a:e] - k * R_CORE - b * 128).astype(np.float16)
                sstream[o:o + n] = s_all[e_row[a:e]]
        # wrapped layouts
        cw = np.tile(colidx.reshape(S16, 16).T, (8, 1))          # [128, S16]
        rw = np.ascontiguousarray(rowloc.reshape(S128, 128).T)   # [128, S128]
        sw = np.ascontiguousarray(
            sstream.reshape(S128, 128, 8).transpose(1, 0, 2).reshape(128, S128 * 8))
        per_core.append(dict(colidx_w=cw, rowloc_w=rw, sst_w=sw))

    shared = dict(xT=xT, WC=WC, iota=iota)
    sched = dict(groups=groups, S=S, S16=S16, S128=S128)
    return shared, per_core, sched


def build_program(sched):
    import concourse.bacc as bacc
    import concourse.bass as bass
    import concourse.mybir as mybir
    import concourse.tile as tile
    from concourse.library_config import mlp

    f16, f32, i16 = mybir.dt.float16, mybir.dt.float32, mybir.dt.int16
    S, S16, S128 = sched["S"], sched["S16"], sched["S128"]
    groups = sched["groups"]

    nc = bacc.Bacc("TRN2", target_bir_lowering=False, debug=False,
                   num_devices=NCORES)
    xT_d = nc.dram_tensor("xT_in", [128, NPAD], f16, kind="ExternalInput").ap()
    wc_d = nc.dram_tensor("wc_in", [128, TUSED], f16, kind="ExternalInput").ap()
    iota_d = nc.dram_tensor("iota_in", [128, 128], f16, kind="ExternalInput").ap()
    ci_d = nc.dram_tensor("colidx_in", [128, S16], i16, kind="ExternalInput").ap()
    rl_d = nc.dram_tensor("rowloc_in", [128, S128], f16, kind="ExternalInput").ap()
    ss_d = nc.dram_tensor("sst_in", [128, S128 * 8], f32, kind="ExternalInput").ap()
    out_d = nc.dram_tensor("o_out", [RPAD, IN_DIM], f32, kind="ExternalOutput").ap()
    tbl_d = nc.dram_tensor("table", [NPAD, TROW], f16, kind="Internal").ap()

    with tile.TileContext(nc) as tc:
        with tc.tile_pool(name="const", bufs=1) as cp:
            wc = cp.tile([128, TUSED], f16)
            iota = cp.tile([128, 128], f16)
            rowloc = cp.tile([128, S128], f16)
            ebias = cp.tile([128, 1], f32)
            nc.vector.memset(ebias[:], EXP_BIAS)
            nc.sync.dma_start(wc[:], wc_d)
            nc.sync.dma_start(iota[:], iota_d)
            nc.sync.dma_start(rowloc[:], rl_d)
            nc.gpsimd.load_library(mlp)

            # ---------- Phase A: projection into the gather table ----------
            with tc.tile_pool(name="pa", bufs=3) as pa, \
                 tc.tile_pool(name="psA", bufs=4, space="PSUM") as psA:
                for Bt in range(NBATCH):
                    xt = pa.tile([128, BATCH_NODES], f16, tag="xt")
                    nc.sync.dma_start(
                        xt[:], xT_d[:, Bt * BATCH_NODES:(Bt + 1) * BATCH_NODES])
                    st = pa.tile([128, TA * TROW], f16, tag="st")
                    st3 = st[:].rearrange("p (t c) -> p t c", c=TROW)
                    for j in range(TA):
                        ps = psA.tile([128, TUSED], f32)
                        nc.tensor.matmul(ps[:], lhsT=xt[:, j * 128:(j + 1) * 128],
                                         rhs=wc[:], start=True, stop=True)
                        if j % 2 == 0:
                            nc.vector.tensor_copy(st3[:, j, 0:TUSED], ps[:])
                        else:
                            nc.scalar.copy(st3[:, j, 0:TUSED], ps[:])
                    # table rows for batch: row n = Bt*3200 + p*25 + j
                    dst = tbl_d[Bt * BATCH_NODES:(Bt + 1) * BATCH_NODES, :] \
                        .rearrange("(p t) c -> p t c", p=128, t=TA)
                    nc.sync.dma_start(dst, st3)

            # ---------- Phase B: edge processing ----------
            with tc.tile_pool(name="pb", bufs=2) as pb, \
                 tc.tile_pool(name="pfl", bufs=4) as pfl, \
                 tc.tile_pool(name="psB", bufs=8, space="PSUM") as psB:
                for g in groups:
                    O, E = g["start"], g["end"]
                    L = E - O
                    if L == 0:
                        continue
                    nsub = L // 128
                    colidx = pb.tile([128, L // 16], i16, tag="colidx")
                    nc.sync.dma_start(colidx[:], ci_d[:, O // 16:E // 16])
                    xpd = pb.tile([128, nsub * TROW], f16, tag="xpd")
                    for (roff, rlen, B) in g["runs"]:
                        if rlen == 0:
                            continue
                        a = roff - O
                        dest = xpd[:, (a // 128) * TROW:((a + rlen) // 128) * TROW] \
                            .rearrange("p (i e) -> p i e", e=TROW)
                        nc.gpsimd.dma_gather(
                            dest, tbl_d[B * BUCKET:(B + 1) * BUCKET, :],
                            colidx[:, a // 16:(a + rlen) // 16],
                            rlen, rlen, TROW, single_packet=False)
                    sst = pb.tile([128, nsub * 8], f32, tag="sst")
                    nc.sync.dma_start(sst[:], ss_d[:, (O // 128) * 8:(O // 128 + nsub) * 8])

                    xpd3 = xpd[:].rearrange("p (t c) -> p t c", c=TROW)
                    sst3 = sst[:].rearrange("p (t h) -> p t h", h=8)
                    tt = pb.tile([128, nsub * 8], f32, tag="tt")
                    tt3 = tt[:].rearrange("p (t h) -> p t h", h=8)
                    nc.vector.tensor_add(tt3, sst3, xpd3[:, :, 128:136])
                    uu = pb.tile([128, nsub * 8], f32, tag="uu")
                    uu3 = uu[:].rearrange("p (t h) -> p t h", h=8)
                    nc.vector.scalar_tensor_tensor(
                        uu3, tt3, NEG_SLOPE, tt3,
                        op0=mybir.AluOpType.mult, op1=mybir.AluOpType.max)
                    rhs = pb.tile([128, nsub * TUSED], f16, tag="rhs")
                    rhs3 = rhs[:].rearrange("p (t c) -> p t c", c=TUSED)
                    nc.scalar.activation(rhs3[:, :, 128:136], uu3,
                                         mybir.ActivationFunctionType.Exp,
                                         bias=ebias[:], scale=1.0)
                    w4 = rhs3[:, :, 128:136].unsqueeze(2).to_broadcast([128, nsub, HD, H])
                    xp4 = xpd3[:, :, 0:128].rearrange("p t (u h) -> p t u h", h=H)
                    msg4 = rhs3[:, :, 0:128].rearrange("p t (u h) -> p t u h", h=H)
                    nc.vector.tensor_mul(msg4, w4, xp4)
                    sel = pb.tile([128, nsub * 128], f16, tag="sel")
                    sel3 = sel[:].rearrange("p (t m) -> p t m", m=128)
                    nc.vector.tensor_tensor(
                        sel3,
                        iota[:].unsqueeze(1).to_broadcast([128, nsub, 128]),
                        rowloc[:, O // 128:O // 128 + nsub].unsqueeze(2)
                            .to_broadcast([128, nsub, 128]),
                        op=mybir.AluOpType.is_equal)

                    # matmuls, one per subtile, accumulated per block
                    subs = g["subtiles"]
                    first = {}
                    last = {}
                    for si, b in enumerate(subs):
                        first.setdefault(b, si)
                        last[b] = si
                    ps_of = {}
                    for si, b in enumerate(subs):
                        if b not in ps_of:
                            ps_of[b] = psB.tile([128, TUSED], f32, tag="psb",
                                                name=f"psb_g{g['start']}_b{b}")
                        nc.tensor.matmul(
                            ps_of[b][:],
                            lhsT=sel[:, si * 128:(si + 1) * 128],
                            rhs=rhs[:, si * TUSED:(si + 1) * TUSED],
                            start=(si == first[b]), stop=(si == last[b]))
                    # flush blocks
                    for b in g["blocks"]:
                        if b not in ps_of:
                            continue
                        ps = ps_of[b]
                        den = pfl.tile([128, 8], f32, tag="den")
                        nc.vector.tensor_scalar(den[:], ps[:, 128:136], 1e-30, None,
                                                op0=mybir.AluOpType.add)
                        rec = pfl.tile([128, 8], f32, tag="rec")
                        nc.vector.reciprocal(rec[:], den[:])
                        ot = pfl.tile([128, IN_DIM], f32, tag="ot")
                        otv = ot[:].rearrange("p (h u) -> p h u", u=HD)
                        psv = ps[:, 0:128].rearrange("p (u h) -> p u h", h=H) \
                            .transpose([0, 2, 1])
                        recv = rec[:].unsqueeze(2).to_broadcast([128, H, HD])
                        nc.vector.tensor_mul(otv, psv, recv)
                        nc.sync.dma_start(out_d[b * 128:(b + 1) * 128, :], ot[:])
    nc.compile()
    return nc


_CACHE = {}


def kernel(x, edge_indices, W, src_attn, dst_attn):
    import concourse.bass_utils as bass_utils

    shared, per_core, sched = host_prep(x, edge_indices, W, src_attn, dst_attn)
    nc = build_program(sched)
    in_maps = []
    for k in range(NCORES):
        in_maps.append({
            "xT_in": shared["xT"], "wc_in": shared["WC"], "iota_in": shared["iota"],
            "colidx_in": per_core[k]["colidx_w"],
            "rowloc_in": per_core[k]["rowloc_w"],
            "sst_in": per_core[k]["sst_w"],
        })
    res = bass_utils.run_bass_kernel_spmd(nc, in_maps, core_ids=list(range(NCORES)))
    out = np.concatenate([res.results[k]["o_out"][:R_CORE] for k in range(NCORES)],
                         axis=0)
    return out
